# revision 3
# baseline (speedup 1.0000x reference)
"""Trainium2 Bass kernel for nn_AffinityBiFC.

Reference computation (B=4, N=M=128, D=256, BD=1024):
    t  = einsum('bnd,dek->bnek', X, A)
    bi = einsum('bnek,bme->bnmk', t, Y)
    S  = einsum('bnmk,ok->bnmo', bi, W) + b        -> S[..., 0]  [B, N, M]

Algebraic collapse (exact reassociation):
    Aw[d, e] = sum_k A[d, e, k] * W[0, k]          # one streaming pass over A (268 MB)
    S[b]     = X[b] @ Aw @ Y[b].T + b              # tiny matmuls

Sharding: A is split over its first (d) axis across the 8 cores (each core
streams a contiguous 33.5 MB block and produces 32 rows of Aw).  The partial
Aw rows are AllGathered (32 KB/rank), then every core redundantly computes the
final small matmuls and writes the full output; the host takes core 0's copy.

On-chip pipeline per core:
  - DMA A in [128, DD, 2, 1024] tiles (partition = e%128): 4 MB contiguous DMAs.
  - DVE tensor_tensor mult (A_tile * W_rep) -> prod; ACT activation(Copy)
    with accum_out sums prod over k -> one column of acc[e%128, ec, dl] per
    (d, ec).  (tensor_tensor_reduce would fuse both but is rejected by the
    neuronx-cc BIR path on this image.)  DVE ~78us + ACT ~73us per core sit
    under the ~95us DMA stream.
  - PE transpose acc -> Aw_local [32, 256]; AllGather -> Aw [256, 256].
  - PE: T^T[b] = (Aw slice)^T-matmuls with X^T, then S[b] = T^T[b]^T @ ...,
    using PE-transposed X^T / Y^T computed up front (overlapped with stream).
"""

import numpy as np

B, N, D, KD = 4, 128, 256, 1024
P = 128
C = 8                  # cores
DL = D // C            # 32 d-rows per core
DD = 4                 # d-rows per DMA  (4 MB per DMA)
G = DL // DD           # DMA groups per core

_cached = {}


def _build_program():
    import concourse.bass as bass
    import concourse.mybir as mybir
    import concourse.tile as tile
    from concourse import bacc
    from concourse.masks import make_identity

    fp32 = mybir.dt.float32

    nc = bacc.Bacc(
        "TRN2",
        target_bir_lowering=False,
        debug=False,
        num_devices=C,
    )

    a_sh = nc.dram_tensor("a_sh", [DL, D, KD], fp32, kind="ExternalInput").ap()
    x_in = nc.dram_tensor("x_in", [B, N, D], fp32, kind="ExternalInput").ap()
    y_in = nc.dram_tensor("y_in", [B, N, D], fp32, kind="ExternalInput").ap()
    w_rep = nc.dram_tensor("w_rep", [P, KD], fp32, kind="ExternalInput").ap()
    out = nc.dram_tensor("out", [B, N, N], fp32, kind="ExternalOutput").ap()

    with tile.TileContext(nc) as tc:
        with (
            tc.tile_pool(name="apool", bufs=3) as apool,
            tc.tile_pool(name="sbuf", bufs=1) as sbuf,
            tc.tile_pool(name="psum", bufs=4, space="PSUM") as psum,
            tc.tile_pool(name="dram", bufs=1, space="DRAM") as dram,
        ):
            w_sb = sbuf.tile([P, KD], fp32)
            nc.sync.dma_start(w_sb[:], w_rep[:])

            ident = sbuf.tile([P, P], fp32)
            make_identity(nc, ident)

            # X / Y with n|m on partitions: [n, b, chunk, d_lo]
            x_sb = sbuf.tile([P, B, 2, P], fp32)
            nc.sync.dma_start(x_sb[:], x_in.rearrange("b n (c p) -> n b c p", p=P))
            y_sb = sbuf.tile([P, B, 2, P], fp32)
            nc.sync.dma_start(y_sb[:], y_in.rearrange("b m (c p) -> m b c p", p=P))

            # PE transposes: xT[d_lo, dc, b, n], yT[e_lo, ec, b, m]
            xT = sbuf.tile([P, 2, B, P], fp32)
            yT = sbuf.tile([P, 2, B, P], fp32)
            for b in range(B):
                for c2 in range(2):
                    pstx = psum.tile([P, P], fp32, tag="ps")
                    nc.tensor.transpose(pstx, x_sb[:, b, c2, :], ident)
                    nc.any.tensor_copy(out=xT[:, c2, b, :], in_=pstx)
                    psty = psum.tile([P, P], fp32, tag="ps")
                    nc.tensor.transpose(psty, y_sb[:, b, c2, :], ident)
                    nc.any.tensor_copy(out=yT[:, c2, b, :], in_=psty)

            # acc[e_lo, ec, dl] = Aw[c*DL + dl, ec*128 + e_lo]
            acc = sbuf.tile([P, 2, DL], fp32)
            scratch = sbuf.tile([P, KD], fp32)

            a_view = a_sh.rearrange("(g dd) (ec p) k -> g p dd ec k", dd=DD, p=P)
            for g in range(G):
                at = apool.tile([P, DD, 2, KD], fp32, tag="a")
                nc.sync.dma_start(at[:], a_view[g])
                for dd in range(DD):
                    for ec in range(2):
                        prod = apool.tile([P, KD], fp32, tag="prod")
                        nc.vector.tensor_tensor(
                            out=prod[:],
                            in0=at[:, dd, ec, :],
                            in1=w_sb,
                            op=mybir.AluOpType.mult,
                        )
                        nc.scalar.activation(
                            out=scratch[:],
                            in_=prod[:],
                            func=mybir.ActivationFunctionType.Copy,
                            accum_out=acc[:, ec, g * DD + dd : g * DD + dd + 1],
                        )

            # local Aw rows: awT[dl, e] via PE transpose of acc
            awT = sbuf.tile([DL, D], fp32)
            for ec in range(2):
                psa = psum.tile([P, P], fp32, tag="ps")
                nc.tensor.transpose(psa[:DL, :], acc[:, ec, :], ident)
                nc.any.tensor_copy(out=awT[:, ec * P : (ec + 1) * P], in_=psa[:DL, :])

            # AllGather local rows -> full Aw [256, 256]
            cc_in = dram.tile([DL, D], fp32)
            cc_out = dram.tile([D, D], fp32, addr_space="Shared")
            nc.sync.dma_start(cc_in[:], awT[:])
            nc.gpsimd.collective_compute(
                "AllGather",
                mybir.AluOpType.bypass,
                replica_groups=[list(range(C))],
                ins=[cc_in.opt()],
                outs=[cc_out.opt()],
            )

            aw_sb = sbuf.tile([P, 2, D], fp32)  # [d_lo, dc, e]
            nc.sync.dma_start(aw_sb[:], cc_out.rearrange("(dc p) e -> p dc e", p=P))

            # final matmuls: T^T[b][ec] = sum_dc Aw[dc,ec]^T-style, then S[b]
            tT = sbuf.tile([P, 2, B, P], fp32)  # [e_lo, ec, b, n]
            s_sb = sbuf.tile([P, B, N], fp32)   # [n, b, m]
            for b in range(B):
                for ec in range(2):
                    psT = psum.tile([P, P], fp32, tag="ps")
                    for dc in range(2):
                        nc.tensor.matmul(
                            psT,
                            lhsT=aw_sb[:, dc, ec * P : (ec + 1) * P],
                            rhs=xT[:, dc, b, :],
                            start=(dc == 0),
                            stop=(dc == 1),
                        )
                    nc.any.tensor_copy(out=tT[:, ec, b, :], in_=psT)
                psS = psum.tile([P, P], fp32, tag="ps")
                for ec in range(2):
                    nc.tensor.matmul(
                        psS,
                        lhsT=tT[:, ec, b, :],
                        rhs=yT[:, ec, b, :],
                        start=(ec == 0),
                        stop=(ec == 1),
                    )
                nc.any.tensor_copy(out=s_sb[:, b, :], in_=psS)

            nc.sync.dma_start(out.rearrange("b n m -> n b m"), s_sb[:])

    nc.compile()
    return nc


def _get_program():
    if "nc" not in _cached:
        _cached["nc"] = _build_program()
    return _cached["nc"]


def _run(X, Y, A, W, b, trace=False, **trace_kwargs):
    from concourse.bass_utils import run_bass_kernel_spmd

    nc = _get_program()

    X = np.ascontiguousarray(X, dtype=np.float32)
    Y = np.ascontiguousarray(Y, dtype=np.float32)
    A = np.ascontiguousarray(A, dtype=np.float32)
    W = np.ascontiguousarray(W, dtype=np.float32)
    w_rep = np.ascontiguousarray(
        np.broadcast_to(W.reshape(1, KD), (P, KD)), dtype=np.float32
    )

    core_ids = list(range(C))
    in_maps = [
        {
            "a_sh": A[c * DL : (c + 1) * DL],
            "x_in": X,
            "y_in": Y,
            "w_rep": w_rep,
        }
        for c in core_ids
    ]

    res = run_bass_kernel_spmd(nc, in_maps, core_ids, trace=trace, **trace_kwargs)
    out = np.asarray(res.results[0]["out"], dtype=np.float32)
    out = out + np.float32(b.reshape(-1)[0])
    return out, res


def kernel(X, Y, A, W, b):
    out, _ = _run(X, Y, A, W, b, trace=False)
    return out


# revision 7
# speedup vs baseline: 1.1460x; 1.1460x over previous
"""Trainium2 Bass kernel for nn_AffinityBiFC.

Reference computation (B=4, N=M=128, D=256, BD=1024):
    t  = einsum('bnd,dek->bnek', X, A)
    bi = einsum('bnek,bme->bnmk', t, Y)
    S  = einsum('bnmk,ok->bnmo', bi, W) + b        -> S[..., 0]  [B, N, M]

Algebraic collapse (exact reassociation):
    Aw[d, e] = sum_k A[d, e, k] * W[0, k]          # one streaming pass over A (268 MB)
    S[b]     = X[b] @ Aw @ Y[b].T + b              # tiny matmuls

Sharding: A is split over its first (d) axis across the 8 cores (each core
streams a contiguous 33.5 MB block and produces 32 rows of Aw).  The partial
Aw rows are AllGathered in two fp16 halves (the first fires mid-stream and is
fully hidden), then every core redundantly computes the final small matmuls
in fp16 (fp32 accumulate) and writes the full output; the host takes core 0's
copy and adds the scalar bias.

Per-core pipeline:
  - DMA A in [128, dd, 2, 1024] tiles (partition = e%128); first groups are
    small so the DVE stream starts early, later groups are 4 MB.
  - DVE tensor_tensor mult (A_tile * W_rep); ACT activation(Copy, accum_out)
    sums over k -> acc[e%128, ec, dl].  DVE ~78us + ACT ~98us sit just under
    the ~100us DMA stream.
  - After 16 d-rows: PE-transpose acc half, cast fp16, AllGather #1 (runs
    during the stream).  After all 32: AllGather #2 (16 rows only).
  - AllGather halves interleave d rows core-major ("comb" order); the final
    matmul contracts over d in the same comb order (sum order is free), with
    X^T transposed per comb half.  S[b] = (Aw^T-chunks x X^T) then x Y^T.
"""

import numpy as np

B, N, D, KD = 4, 128, 256, 1024
P = 128
C = 8                  # cores
DL = D // C            # 32 d-rows per core
HL = DL // 2           # 16 rows per collective half
GROUPS = [1, 1, 2, 4, 4, 4, 4, 4, 4, 4]   # d-rows per DMA; [:6] sums to 16
assert sum(GROUPS) == DL and sum(GROUPS[:6]) == HL

_cached = {}


def _build_program():
    import concourse.bass as bass
    import concourse.mybir as mybir
    import concourse.tile as tile
    from concourse import bacc
    from concourse.masks import make_identity

    fp32 = mybir.dt.float32
    fp16 = mybir.dt.float16

    nc = bacc.Bacc(
        "TRN2",
        target_bir_lowering=False,
        debug=False,
        num_devices=C,
    )

    a_sh = nc.dram_tensor("a_sh", [DL, D, KD], fp32, kind="ExternalInput").ap()
    x_in = nc.dram_tensor("x_in", [B, N, D], fp32, kind="ExternalInput").ap()
    y_in = nc.dram_tensor("y_in", [B, N, D], fp32, kind="ExternalInput").ap()
    w_rep = nc.dram_tensor("w_rep", [P, KD], fp32, kind="ExternalInput").ap()
    out = nc.dram_tensor("out", [B, N, N], fp32, kind="ExternalOutput").ap()

    with tile.TileContext(nc) as tc:
        with (
            tc.tile_pool(name="apool", bufs=4) as apool,
            tc.tile_pool(name="ppool", bufs=3) as ppool,
            tc.tile_pool(name="sbuf", bufs=1) as sbuf,
            tc.tile_pool(name="psum", bufs=4, space="PSUM") as psum,
            tc.tile_pool(name="dram", bufs=1, space="DRAM") as dram,
        ):
            # W first: the stream needs it immediately.
            w_sb = sbuf.tile([P, KD], fp32)
            nc.sync.dma_start(w_sb[:], w_rep[:])

            # acc[e_lo, ec, dl] = Aw[c*DL + dl, ec*128 + e_lo]
            acc = sbuf.tile([P, 2, DL], fp32)
            scratch = sbuf.tile([P, KD], fp32)

            ident = sbuf.tile([P, P], fp32)
            make_identity(nc, ident)

            awT1 = sbuf.tile([HL, D], fp16)
            awT2 = sbuf.tile([HL, D], fp16)
            cc1_in = dram.tile([HL, D], fp16)
            cc2_in = dram.tile([HL, D], fp16)
            cc1_out = dram.tile([P, D], fp16, addr_space="Shared")
            cc2_out = dram.tile([P, D], fp16, addr_space="Shared")

            def flush_half(half, awT, cc_in, cc_out):
                lo = half * HL
                for ec in range(2):
                    psa = psum.tile([P, P], fp32, tag="ps", name=f"psa{half}{ec}")
                    nc.tensor.transpose(psa[:HL, :], acc[:, ec, lo : lo + HL], ident)
                    nc.any.tensor_copy(
                        out=awT[:, ec * P : (ec + 1) * P], in_=psa[:HL, :]
                    )
                nc.sync.dma_start(cc_in[:], awT[:])
                nc.gpsimd.collective_compute(
                    "AllGather",
                    mybir.AluOpType.bypass,
                    replica_groups=[list(range(C))],
                    ins=[cc_in.opt()],
                    outs=[cc_out.opt()],
                )

            # main stream: A groups (all triggers early in program order)
            a_flat = a_sh.rearrange("dl (ec p) k -> p dl ec k", p=P)
            dl0 = 0
            for g, dd in enumerate(GROUPS):
                at = apool.tile([P, 4, 2, KD], fp32, tag="a", name=f"at{g}")
                nc.sync.dma_start(
                    at[:, :dd, :, :], a_flat[:, dl0 : dl0 + dd, :, :]
                )
                for j in range(dd):
                    dl = dl0 + j
                    for ec in range(2):
                        prod = ppool.tile([P, KD], fp32, tag="prod", name=f"pr{dl}{ec}")
                        nc.vector.tensor_tensor(
                            out=prod[:],
                            in0=at[:, j, ec, :],
                            in1=w_sb,
                            op=mybir.AluOpType.mult,
                        )
                        nc.scalar.activation(
                            out=scratch[:],
                            in_=prod[:],
                            func=mybir.ActivationFunctionType.Copy,
                            accum_out=acc[:, ec, dl : dl + 1],
                        )
                dl0 += dd
                if dl0 == HL and dd == GROUPS[5]:
                    flush_half(0, awT1, cc1_in, cc1_out)
            flush_half(1, awT2, cc2_in, cc2_out)

            # X / Y (after the A triggers; only needed late)
            x_sb = sbuf.tile([P, B, D], fp32)   # [n, b, d]
            nc.sync.dma_start(x_sb[:], x_in.rearrange("b n d -> n b d"))
            y_sb = sbuf.tile([P, B, D], fp32)   # [m, b, e]
            nc.sync.dma_start(y_sb[:], y_in.rearrange("b m e -> m b e"))

            # X^T in comb order (matching the AllGather halves):
            #   comb index r = c*16+dl  <->  d = c*32 + h*16 + dl
            xT = sbuf.tile([P, 2, B, P], fp16)  # [comb, h, b, n]
            for b in range(B):
                x_comb = x_sb[:, b].rearrange(
                    "n (c h dl) -> n h c dl", c=C, h=2, dl=HL
                )
                for h in range(2):
                    # gather the comb columns contiguously first: the PE
                    # transpose's moving operand allows only one free dim
                    xstage = ppool.tile([P, P], fp32, tag="xstage", name=f"xs{b}{h}")
                    nc.vector.tensor_copy(out=xstage[:], in_=x_comb[:, h])
                    pstx = psum.tile([P, P], fp32, tag="ps", name=f"pstx{b}{h}")
                    nc.tensor.transpose(pstx, xstage[:], ident)
                    nc.any.tensor_copy(out=xT[:, h, b, :], in_=pstx)

            # Y^T in natural e-chunk order
            yT = sbuf.tile([P, 2, B, P], fp16)  # [e_lo, ec, b, m]
            for b in range(B):
                for ec in range(2):
                    psty = psum.tile([P, P], fp32, tag="ps", name=f"psty{b}{ec}")
                    nc.tensor.transpose(
                        psty, y_sb[:, b, ec * P : (ec + 1) * P], ident
                    )
                    nc.any.tensor_copy(out=yT[:, ec, b, :], in_=psty)

            # Aw comb halves from the collectives
            g1_sb = sbuf.tile([P, D], fp16)
            nc.sync.dma_start(g1_sb[:], cc1_out[:])
            g2_sb = sbuf.tile([P, D], fp16)
            nc.sync.dma_start(g2_sb[:], cc2_out[:])

            # final matmuls: T^T[b][ec] = sum_comb Aw^T x X^T, then S[b]
            tT = sbuf.tile([P, 2, B, P], fp16)  # [e_lo, ec, b, n]
            s_sb = sbuf.tile([P, B, N], fp32)   # [n, b, m]
            for b in range(B):
                for ec in range(2):
                    psT = psum.tile([P, P], fp32, tag="ps", name=f"psT{b}{ec}")
                    nc.tensor.matmul(
                        psT,
                        lhsT=g1_sb[:, ec * P : (ec + 1) * P],
                        rhs=xT[:, 0, b, :],
                        start=True,
                        stop=False,
                    )
                    nc.tensor.matmul(
                        psT,
                        lhsT=g2_sb[:, ec * P : (ec + 1) * P],
                        rhs=xT[:, 1, b, :],
                        start=False,
                        stop=True,
                    )
                    nc.any.tensor_copy(out=tT[:, ec, b, :], in_=psT)
                psS = psum.tile([P, P], fp32, tag="ps", name=f"psS{b}")
                for ec in range(2):
                    nc.tensor.matmul(
                        psS,
                        lhsT=tT[:, ec, b, :],
                        rhs=yT[:, ec, b, :],
                        start=(ec == 0),
                        stop=(ec == 1),
                    )
                nc.any.tensor_copy(out=s_sb[:, b, :], in_=psS)

            nc.sync.dma_start(out.rearrange("b n m -> n b m"), s_sb[:])

    nc.compile()
    return nc


def _get_program():
    if "nc" not in _cached:
        _cached["nc"] = _build_program()
    return _cached["nc"]


def _run(X, Y, A, W, b, trace=False, **trace_kwargs):
    from concourse.bass_utils import run_bass_kernel_spmd

    nc = _get_program()

    X = np.ascontiguousarray(X, dtype=np.float32)
    Y = np.ascontiguousarray(Y, dtype=np.float32)
    A = np.ascontiguousarray(A, dtype=np.float32)
    W = np.ascontiguousarray(W, dtype=np.float32)
    w_rep = np.ascontiguousarray(
        np.broadcast_to(W.reshape(1, KD), (P, KD)), dtype=np.float32
    )

    core_ids = list(range(C))
    in_maps = [
        {
            "a_sh": A[c * DL : (c + 1) * DL],
            "x_in": X,
            "y_in": Y,
            "w_rep": w_rep,
        }
        for c in core_ids
    ]

    res = run_bass_kernel_spmd(nc, in_maps, core_ids, trace=trace, **trace_kwargs)
    out = np.asarray(res.results[0]["out"], dtype=np.float32)
    out = out + np.float32(b.reshape(-1)[0])
    return out, res


def kernel(X, Y, A, W, b):
    out, _ = _run(X, Y, A, W, b, trace=False)
    return out
